# revision 1
# baseline (speedup 1.0000x reference)
"""Trainium2 Bass kernel for nn_CrossGraphNetLite (dual-GNN + gated fusion + classifier).

Strategy (8 NeuronCores, graph/data parallel):
  * Host preprocesses the *integer* graph structure only (edge lists, types,
    batch assignment, degrees) into dense coefficient matrices:
      - Layer 1 per dst-node-block k:  T_k[t, v] = sum of edge coeffs into v
        bucketed by source-node *type* t (+ self-loop + bias row). Then
        x1 = relu(emb_aug^T T) with emb_aug = [emb @ W1; b1] computed on device.
      - Layer 2 + mean-pool collapse:  C_k[s, g] = sum of edge coeffs from
        src s (core k's block) into any node of graph g (+ self-loop).
        pool^T = sum_s h2[s]^T C[s, :], accumulated on the PE into PSUM.
  * Per-core partial pools are exchanged with a single AllToAll and summed
    locally; a tiny per-core epilogue does gated fusion, the semantic MLP,
    LayerNorm and the classifier for that core's 128 graphs.
  * Stream path (T, C, x2, h2) is fp16; all accumulation is fp32 on PSUM.
  * The h2-production and pool-accumulation loops for BOTH graph types are
    fused and interleaved so the PE stays dense (HAM stays un-throttled)
    while the C matrices stream at full HBM rate.
"""

import sys

sys.path.insert(0, "/opt/trn_rl_repo")

import numpy as np

import concourse.bacc as bacc
import concourse.bass as bass
import concourse.mybir as mybir
import concourse.tile as tile

AF = mybir.ActivationFunctionType
ALU = mybir.AluOpType
F32 = mybir.dt.float32
F16 = mybir.dt.float16


class CFG:
    def __init__(self, n=100000, e=1250000, g=1024, ncores=8,
                 nta=200, ntc=100, sem=768, sdt=mybir.dt.float16):
        self.N = n
        self.E = e
        self.G = g
        self.NCORES = ncores
        self.NTA = nta          # ast types
        self.NTC = ntc          # cfg types
        self.SEM = sem
        self.sdt = sdt          # stream dtype for T / C / x2 / h2
        self.NB = n // ncores                      # nodes per core
        self.NBP = ((self.NB + 511) // 512) * 512  # padded nodes per core
        self.NCHUNK = self.NBP // 512
        self.NSB = self.NBP // 128                 # 128-node src blocks
        self.GB = g // ncores                      # graphs per core
        self.TPA = 256                             # ast types padded (+bias row @ nta)
        self.TPC = 128                             # cfg types padded (+bias row @ ntc)
        assert nta + 1 <= self.TPA and ntc + 1 <= self.TPC
        self.GSLICES = [(i, min(i + 512, g)) for i in range(0, g, 512)]
        self.SEMK = sem // 128


def _np_sdt(sdt):
    return {mybir.dt.float16: np.float16,
            mybir.dt.float32: np.float32}[sdt]


def build_nc(cfg: CFG):
    """Build the SPMD Bass program (identical on all cores; per-core data differs)."""
    nc = bacc.Bacc("TRN2", target_bir_lowering=False, debug=False,
                   enable_asserts=True, num_devices=cfg.NCORES)
    sdt = cfg.sdt
    G, GB, NBP = cfg.G, cfg.GB, cfg.NBP

    def din(name, shape, dt=F32):
        return nc.dram_tensor(name, list(shape), dt, kind="ExternalInput").ap()

    T_ast = din("T_ast", [cfg.TPA, NBP], sdt)
    T_cfg = din("T_cfg", [cfg.TPC, NBP], sdt)
    C_ast = din("C_ast", [NBP, G], sdt)
    C_cfg = din("C_cfg", [NBP, G], sdt)
    embT_ast = din("embT_ast", [64, cfg.TPA])
    embT_cfg = din("embT_cfg", [64, cfg.TPC])
    astW1 = din("astW1", [64, 64])
    cfgW1 = din("cfgW1", [64, 64])
    astb1s = din("astb1s", [1, 64], sdt)
    cfgb1s = din("cfgb1s", [1, 64], sdt)
    astW2 = din("astW2", [64, 64])
    cfgW2 = din("cfgW2", [64, 64])
    astb2 = din("astb2", [1, 64])
    cfgb2 = din("cfgb2", [1, 64])
    cnt_ast = din("cnt_ast", [1, G])    # node counts masked to this core's graphs
    cnt_cfg = din("cnt_cfg", [1, G])
    Wg1 = din("Wg1", [128, 64])
    bg1c = din("bg1c", [64, 1])
    Wsem = din("Wsem", [cfg.SEM, 64])
    bsemc = din("bsemc", [64, 1])
    Wg2 = din("Wg2", [128, 64])
    bg2c = din("bg2c", [64, 1])
    lngc = din("lngc", [64, 1])
    lnbc = din("lnbc", [64, 1])
    Wc = din("Wc", [64, 2])
    bcc = din("bcc", [2, 1])
    semT = din("semT", [cfg.SEM, GB])
    out_ap = nc.dram_tensor("outT", [2, GB], F32, kind="ExternalOutput").ap()

    with tile.TileContext(nc) as tc:
        with (
            tc.tile_pool(name="consts", bufs=1) as consts,
            tc.tile_pool(name="x2t", bufs=1) as x2t_pool,
            tc.tile_pool(name="h2p", bufs=4) as h2_pool,
            tc.tile_pool(name="tstream", bufs=6) as tstream,
            tc.tile_pool(name="cstream", bufs=20) as cstream,
            tc.tile_pool(name="small", bufs=1) as small,
            tc.tile_pool(name="ps_x1", bufs=2, space="PSUM") as ps_x1,
            tc.tile_pool(name="ps_h2", bufs=2, space="PSUM") as ps_h2,
            tc.tile_pool(name="ps_pool", bufs=1, space="PSUM") as ps_pool,
            tc.tile_pool(name="dram", bufs=1, space="DRAM") as dram,
        ):
            # ---- small constants: load on the GPSIMD (SWDGE) ring so the
            # sync HWDGE ring starts streaming T/C immediately ----
            def load_const(ap, shape, dt=F32, name=None):
                t = consts.tile(list(shape), dt, name=name or ap.tensor.name + "_sb")
                nc.gpsimd.dma_start(t[:], ap[:])
                return t

            # critical consts only — everything the stream path needs
            embT_ast_sb = load_const(embT_ast, [64, cfg.TPA])
            embT_cfg_sb = load_const(embT_cfg, [64, cfg.TPC])
            astW1_sb = load_const(astW1, [64, 64])
            cfgW1_sb = load_const(cfgW1, [64, 64])
            astW2_sb = load_const(astW2, [64, 64])
            cfgW2_sb = load_const(cfgW2, [64, 64])
            astb2_sb = load_const(astb2, [1, 64])
            cfgb2_sb = load_const(cfgb2, [1, 64])
            cnt_ast_sb = load_const(cnt_ast, [1, G])
            cnt_cfg_sb = load_const(cnt_cfg, [1, G])
            # fp16 copies of the W2 weights for the fp16 h2 matmuls
            astW2h = consts.tile([64, 64], sdt, name="astW2h")
            nc.vector.tensor_copy(astW2h[:], astW2_sb[:])
            cfgW2h = consts.tile([64, 64], sdt, name="cfgW2h")
            nc.vector.tensor_copy(cfgW2h[:], cfgW2_sb[:])

            # ---- phase A: emb_aug tables (emb @ W1 with bias row) ----
            def build_emb_aug(embT_sb, W1_sb, b1s_ap, tp, nt, tag):
                tiles = []
                for i in range(tp // 128):
                    ps = ps_h2.tile([128, 64], F32, name=f"psA_{tag}{i}", tag="ph")
                    nc.tensor.matmul(ps[:], embT_sb[:, i * 128:(i + 1) * 128],
                                     W1_sb[:], start=True, stop=True)
                    ea = small.tile([128, 64], sdt, name=f"ea_{tag}{i}")
                    nc.vector.tensor_copy(ea[:], ps[:])
                    tiles.append(ea)
                # overwrite bias row (type index nt)
                bi, br = divmod(nt, 128)
                nc.gpsimd.dma_start(tiles[bi][br:br + 1, :], b1s_ap[:])
                return tiles

            ea_ast = build_emb_aug(embT_ast_sb, astW1_sb, astb1s, cfg.TPA, cfg.NTA, "a")
            ea_cfg = build_emb_aug(embT_cfg_sb, cfgW1_sb, cfgb1s, cfg.TPC, cfg.NTC, "c")

            # ---- fused streaming loop: per 512-node chunk build x2T slice
            # (layer 1), then produce h2 for its 4 sub-blocks in one PSUM bank
            # and accumulate both pools.  C streams on both HWDGE rings.
            # Pool accumulation is split in halves so the first AllToAll can
            # overlap the second half of the stream. ----
            x2T_ast = x2t_pool.tile([64, NBP], sdt, name="x2T_a", tag="x2T_a")
            x2T_cfg = x2t_pool.tile([64, NBP], sdt, name="x2T_c", tag="x2T_c")
            pool_ast = ps_pool.tile([64, G], F32, name="pool_ast")
            pool_cfg = ps_pool.tile([64, G], F32, name="pool_cfg")
            SPLITS = (cfg.NSB - 1,)          # last s of the accumulation
            RESTART = None

            def bstep(c, T_ap, tp, ea_tiles, x2T, tag):
                sl = slice(c * 512, (c + 1) * 512)
                px = ps_x1.tile([64, 512], F32, name=f"px_{tag}{c}", tag="px")
                for i in range(tp // 128):
                    tt = tstream.tile([128, 512], sdt,
                                      name=f"t_{tag}{c}_{i}", tag="tstream")
                    nc.gpsimd.dma_start(tt[:], T_ap[i * 128:(i + 1) * 128, sl])
                    nc.tensor.matmul(px[:], ea_tiles[i][:], tt[:],
                                     start=(i == 0), stop=(i == tp // 128 - 1))
                nc.scalar.activation(x2T[:, sl], px[:], AF.Relu)

            def h2step(c):
                # h2 for the 4 sub-blocks of chunk c, BOTH graph types, in one
                # PSUM bank with a single DVE evacuation (ast cols 0:256,
                # cfg cols 256:512)
                ph8 = ps_h2.tile([128, 512], F32, name=f"ph_{c}", tag="ph")
                for j, (x2T, W2h) in enumerate(((x2T_ast, astW2h),
                                                (x2T_cfg, cfgW2h))):
                    for i in range(4):
                        s = 4 * c + i
                        nc.tensor.matmul(ph8[:, j * 256 + i * 64:j * 256 + (i + 1) * 64],
                                         x2T[:, s * 128:(s + 1) * 128],
                                         W2h[:], start=True, stop=True)
                h2q = h2_pool.tile([128, 512], sdt, name=f"h2_{c}", tag="h2")
                nc.vector.tensor_copy(h2q[:], ph8[:])
                return h2q

            def poolstep(c, h2q, joff, C_ap, pool_ps, tag, dma_eng):
                for i in range(4):
                    s = 4 * c + i
                    ct = cstream.tile([128, G], sdt, name=f"c_{tag}{s}", tag="cstream")
                    dma_eng.dma_start(ct[:], C_ap[s * 128:(s + 1) * 128, :])
                    for (g0, g1) in cfg.GSLICES:
                        nc.tensor.matmul(pool_ps[:, g0:g1],
                                         h2q[:, joff + i * 64:joff + (i + 1) * 64],
                                         ct[:, g0:g1],
                                         start=(s == RESTART),
                                         stop=(s in SPLITS))

            def pool_flush(idx):
                # evacuate both pools (fp16) and exchange via AllToAll; chunk k
                # of the output holds core k's partials for this core's graphs.
                pA = small.tile([64, G], sdt, name=f"pA{idx}")
                pC = small.tile([64, G], sdt, name=f"pC{idx}")
                nc.vector.tensor_copy(pA[:], pool_ast[:])
                nc.vector.tensor_copy(pC[:], pool_cfg[:])
                a_in = dram.tile([cfg.NCORES, 64, 2 * GB], sdt, name=f"a2a_in{idx}")
                a_out = dram.tile([cfg.NCORES, 64, 2 * GB], sdt, name=f"a2a_out{idx}")
                nc.sync.dma_start(
                    a_in[:, :, 0:GB].rearrange("j p d -> p j d"),
                    pA[:].rearrange("p (j d) -> p j d", j=cfg.NCORES))
                nc.scalar.dma_start(
                    a_in[:, :, GB:2 * GB].rearrange("j p d -> p j d"),
                    pC[:].rearrange("p (j d) -> p j d", j=cfg.NCORES))
                nc.gpsimd.collective_compute(
                    "AllToAll", ALU.bypass,
                    replica_groups=[list(range(cfg.NCORES))],
                    ins=[a_in.opt()], outs=[a_out.opt()])
                return a_out

            # count * b2 folded in as the first (K=1, fp32) accumulating matmul
            for (g0, g1) in cfg.GSLICES:
                nc.tensor.matmul(pool_ast[:, g0:g1], astb2_sb[:],
                                 cnt_ast_sb[:, g0:g1], start=True, stop=False)
                nc.tensor.matmul(pool_cfg[:, g0:g1], cfgb2_sb[:],
                                 cnt_cfg_sb[:, g0:g1], start=True, stop=False)
            # 1-chunk software pipeline: emit next chunk's layer-1 matmuls
            # between this chunk's h2 production and pool accumulation so the
            # PE never head-of-line stalls on the ACT/DVE round trips.
            a2a_outs = []
            bstep(0, T_ast, cfg.TPA, ea_ast, x2T_ast, "a")
            bstep(0, T_cfg, cfg.TPC, ea_cfg, x2T_cfg, "c")
            for c in range(cfg.NCHUNK):
                h2q = h2step(c)
                if c + 1 < cfg.NCHUNK:
                    bstep(c + 1, T_ast, cfg.TPA, ea_ast, x2T_ast, "a")
                    bstep(c + 1, T_cfg, cfg.TPC, ea_cfg, x2T_cfg, "c")
                poolstep(c, h2q, 0, C_ast, pool_ast, "a", nc.sync)
                poolstep(c, h2q, 256, C_cfg, pool_cfg, "c", nc.scalar)

            # ---- deferred consts + semantic branch (overlaps stream tail) ----
            Wg1_sb = load_const(Wg1, [128, 64])
            bg1_sb = load_const(bg1c, [64, 1])
            bsem_sb = load_const(bsemc, [64, 1])
            Wg2_sb = load_const(Wg2, [128, 64])
            bg2_sb = load_const(bg2c, [64, 1])
            lng_sb = load_const(lngc, [64, 1])
            lnb_sb = load_const(lnbc, [64, 1])
            Wc_sb = load_const(Wc, [64, 2])
            bcc_sb = load_const(bcc, [2, 1])
            Wsem_sb = consts.tile([128, cfg.SEMK * 64], F32)
            semT_sb = consts.tile([128, cfg.SEMK * GB], F32)
            for kc in range(cfg.SEMK):
                nc.gpsimd.dma_start(Wsem_sb[:, kc * 64:(kc + 1) * 64],
                                    Wsem[kc * 128:(kc + 1) * 128, :])
                nc.gpsimd.dma_start(semT_sb[:, kc * GB:(kc + 1) * GB],
                                    semT[kc * 128:(kc + 1) * 128, :])
            pssem = ps_x1.tile([64, GB], F32, name="pssem", tag="px")
            for kc in range(cfg.SEMK):
                nc.tensor.matmul(pssem[:],
                                 Wsem_sb[:, kc * 64:(kc + 1) * 64],
                                 semT_sb[:, kc * GB:(kc + 1) * GB],
                                 start=(kc == 0), stop=(kc == cfg.SEMK - 1))
            hsem = small.tile([64, GB], F32, name="hsem")
            nc.scalar.activation(hsem[:], pssem[:], AF.Relu, bias=bsem_sb[:])

            a2a_outs.append(pool_flush(1))

            # ---- phase F: local sum + epilogue for this core's GB graphs ----
            W = 2 * GB
            NCH = len(a2a_outs) * cfg.NCORES
            acc = small.tile([64, NCH * W], sdt, name="acc")
            for hi, a_out in enumerate(a2a_outs):
                nc.sync.dma_start(
                    acc[:, hi * cfg.NCORES * W:(hi + 1) * cfg.NCORES * W]
                    .rearrange("p (j d) -> p j d", j=cfg.NCORES),
                    a_out[:, :, :].rearrange("j p d -> p j d"))
            span = NCH
            while span > 2:
                half = span // 2
                for i in range(half):
                    nc.vector.tensor_add(
                        acc[:, i * W:(i + 1) * W],
                        acc[:, i * W:(i + 1) * W],
                        acc[:, (i + half) * W:(i + half + 1) * W])
                span = half
            accf = small.tile([64, W], F32, name="accf")
            nc.vector.tensor_add(accf[:], acc[:, 0:W], acc[:, W:2 * W])
            hA = accf[:, 0:GB]
            hC = accf[:, GB:2 * GB]

            # gated fuse 1: g = sigmoid([hA, hC] @ Wg1 + bg1); hs = hC + g*(hA-hC)
            cat1 = small.tile([128, GB], F32, name="cat1")
            nc.gpsimd.dma_start(cat1[0:64, :], hA)
            nc.gpsimd.dma_start(cat1[64:128, :], hC)
            psg1 = ps_x1.tile([64, GB], F32, name="psg1", tag="px")
            nc.tensor.matmul(psg1[:], Wg1_sb[:], cat1[:], start=True, stop=True)
            g1 = small.tile([64, GB], F32, name="g1")
            nc.scalar.activation(g1[:], psg1[:], AF.Sigmoid, bias=bg1_sb[:])
            d1 = small.tile([64, GB], F32, name="d1")
            nc.vector.tensor_sub(d1[:], hA, hC)
            t1 = small.tile([64, GB], F32, name="t1")
            nc.vector.tensor_mul(t1[:], g1[:], d1[:])
            hs = small.tile([64, GB], F32, name="hs")
            nc.vector.tensor_add(hs[:], hC, t1[:])

            # gated fuse 2 with the (precomputed) semantic branch
            cat2 = small.tile([128, GB], F32, name="cat2")
            nc.gpsimd.dma_start(cat2[0:64, :], hs[:])
            nc.gpsimd.dma_start(cat2[64:128, :], hsem[:])
            psg2 = ps_x1.tile([64, GB], F32, name="psg2", tag="px")
            nc.tensor.matmul(psg2[:], Wg2_sb[:], cat2[:], start=True, stop=True)
            g2 = small.tile([64, GB], F32, name="g2")
            nc.scalar.activation(g2[:], psg2[:], AF.Sigmoid, bias=bg2_sb[:])
            d2 = small.tile([64, GB], F32, name="d2")
            nc.vector.tensor_sub(d2[:], hs[:], hsem[:])
            t2 = small.tile([64, GB], F32, name="t2")
            nc.vector.tensor_mul(t2[:], g2[:], d2[:])
            h = small.tile([64, GB], F32, name="hfin")
            nc.vector.tensor_add(h[:], hsem[:], t2[:])

            # LayerNorm over the 64 features (partition axis) via matmul reduce
            ones64 = small.tile([64, 1], F32, name="ones64")
            nc.vector.memset(ones64[:], 1.0 / 64.0)
            ones1 = small.tile([1, 64], F32, name="ones1")
            nc.vector.memset(ones1[:], 1.0)
            psmu = ps_h2.tile([1, GB], F32, name="psmu", tag="ph")
            nc.tensor.matmul(psmu[:], ones64[:], h[:], start=True, stop=True)
            hsq = small.tile([64, GB], F32, name="hsq")
            nc.vector.tensor_mul(hsq[:], h[:], h[:])
            psmsq = ps_h2.tile([1, GB], F32, name="psmsq", tag="ph")
            nc.tensor.matmul(psmsq[:], ones64[:], hsq[:], start=True, stop=True)
            mu_sb = small.tile([1, GB], F32, name="mu_sb")
            nc.vector.tensor_copy(mu_sb[:], psmu[:])
            mu2 = small.tile([1, GB], F32, name="mu2")
            nc.vector.tensor_mul(mu2[:], mu_sb[:], mu_sb[:])
            var = small.tile([1, GB], F32, name="var")
            nc.vector.tensor_sub(var[:], psmsq[:], mu2[:])
            nc.vector.tensor_scalar_add(var[:], var[:], 1e-5)
            sdrow = small.tile([1, GB], F32, name="sdrow")
            nc.scalar.activation(sdrow[:], var[:], AF.Sqrt)
            srow = small.tile([1, GB], F32, name="srow")
            nc.vector.reciprocal(srow[:], sdrow[:])
            trow = small.tile([1, GB], F32, name="trow")
            nc.vector.tensor_mul(trow[:], mu_sb[:], srow[:])
            nc.vector.tensor_scalar_mul(trow[:], trow[:], -1.0)
            pssB = ps_x1.tile([64, GB], F32, name="pssB", tag="px")
            nc.tensor.matmul(pssB[:], ones1[:], srow[:], start=True, stop=True)
            pstB = ps_h2.tile([64, GB], F32, name="pstB", tag="ph")
            nc.tensor.matmul(pstB[:], ones1[:], trow[:], start=True, stop=True)
            hn = small.tile([64, GB], F32, name="hn")
            nc.vector.tensor_mul(hn[:], h[:], pssB[:])
            nc.vector.tensor_add(hn[:], hn[:], pstB[:])
            nc.vector.tensor_scalar(hn[:], hn[:], lng_sb[:], lnb_sb[:],
                                    ALU.mult, ALU.add)
            psout = ps_x1.tile([2, GB], F32, name="psout", tag="px")
            nc.tensor.matmul(psout[:], Wc_sb[:], hn[:], start=True, stop=True)
            outT_sb = small.tile([2, GB], F32, name="outT_sb")
            nc.vector.tensor_scalar_add(outT_sb[:], psout[:], bcc_sb[:])
            nc.sync.dma_start(out_ap[:], outT_sb[:])

    nc.compile()
    return nc


# ---------------------------------------------------------------------------
# host-side preprocessing
# ---------------------------------------------------------------------------

def preprocess(inputs: dict, cfg: CFG):
    """Build the 8 per-core input maps from the full problem inputs."""
    np_sdt = _np_sdt(cfg.sdt)
    N, G, NB, NBP, GB = cfg.N, cfg.G, cfg.NB, cfg.NBP, cfg.GB

    def graph_structs(edge, types, batch, tp, nt):
        src = np.asarray(edge[0], np.int64)
        dst = np.asarray(edge[1], np.int64)
        types = np.asarray(types, np.int64)
        batch = np.asarray(batch, np.int64)
        deg = (np.bincount(dst, minlength=N) + 1.0).astype(np.float32)
        dinv = (1.0 / np.sqrt(deg)).astype(np.float32)
        coeff = (dinv[src] * dinv[dst]).astype(np.float32)
        selfc = (dinv * dinv).astype(np.float32)
        t_src = types[src]
        g_dst = batch[dst]
        Ts, Cs, cnts = [], [], []
        counts = np.bincount(batch, minlength=G).astype(np.float32)
        for k in range(cfg.NCORES):
            lo, hi = k * NB, (k + 1) * NB
            # layer-1 T (sharded by dst block)
            m = (dst >= lo) & (dst < hi)
            flat = t_src[m] * NBP + (dst[m] - lo)
            T = np.bincount(flat, weights=coeff[m].astype(np.float64),
                            minlength=tp * NBP)
            blk = np.arange(lo, hi)
            flat_self = types[blk] * NBP + (blk - lo)
            T += np.bincount(flat_self, weights=selfc[blk].astype(np.float64),
                             minlength=tp * NBP)
            T = T.reshape(tp, NBP)
            T[nt, 0:NB] = 1.0   # bias row
            Ts.append(T.astype(np_sdt))
            # layer-2 C (sharded by src block)
            m2 = (src >= lo) & (src < hi)
            flat2 = (src[m2] - lo) * G + g_dst[m2]
            C = np.bincount(flat2, weights=coeff[m2].astype(np.float64),
                            minlength=NBP * G)
            flat2s = (blk - lo) * G + batch[blk]
            C += np.bincount(flat2s, weights=selfc[blk].astype(np.float64),
                             minlength=NBP * G)
            Cs.append(C.reshape(NBP, G).astype(np_sdt))
            cm = np.zeros((1, G), np.float32)
            cm[0, k * GB:(k + 1) * GB] = counts[k * GB:(k + 1) * GB]
            cnts.append(cm)
        return Ts, Cs, cnts

    Ta, Ca, cnta = graph_structs(inputs["ast_edge"], inputs["ast_type"],
                                 inputs["ast_batch"], cfg.TPA, cfg.NTA)
    Tc, Cc, cntc = graph_structs(inputs["cfg_edge"], inputs["cfg_type"],
                                 inputs["cfg_batch"], cfg.TPC, cfg.NTC)

    f32 = lambda x: np.ascontiguousarray(np.asarray(x, np.float32))
    embT_ast = np.zeros((64, cfg.TPA), np.float32)
    embT_ast[:, 0:cfg.NTA] = f32(inputs["ast_emb"]).T
    embT_cfg = np.zeros((64, cfg.TPC), np.float32)
    embT_cfg[:, 0:cfg.NTC] = f32(inputs["cfg_emb"]).T
    semT = f32(inputs["struct_sem"]).T.copy()  # [SEM, G]

    shared = {
        "embT_ast": embT_ast, "embT_cfg": embT_cfg,
        "astW1": f32(inputs["ast_W1"]), "cfgW1": f32(inputs["cfg_W1"]),
        "astb1s": f32(inputs["ast_b1"]).reshape(1, 64).astype(np_sdt),
        "cfgb1s": f32(inputs["cfg_b1"]).reshape(1, 64).astype(np_sdt),
        "astW2": f32(inputs["ast_W2"]), "cfgW2": f32(inputs["cfg_W2"]),
        "astb2": f32(inputs["ast_b2"]).reshape(1, 64),
        "cfgb2": f32(inputs["cfg_b2"]).reshape(1, 64),
        "Wg1": f32(inputs["Wg1"]), "bg1c": f32(inputs["bg1"]).reshape(64, 1),
        "Wsem": f32(inputs["Wsem"]), "bsemc": f32(inputs["bsem"]).reshape(64, 1),
        "Wg2": f32(inputs["Wg2"]), "bg2c": f32(inputs["bg2"]).reshape(64, 1),
        "lngc": f32(inputs["ln_g"]).reshape(64, 1),
        "lnbc": f32(inputs["ln_b"]).reshape(64, 1),
        "Wc": f32(inputs["Wc"]), "bcc": f32(inputs["bc"]).reshape(2, 1),
    }
    in_maps = []
    for k in range(cfg.NCORES):
        m = dict(shared)
        m["T_ast"] = Ta[k]
        m["T_cfg"] = Tc[k]
        m["C_ast"] = Ca[k]
        m["C_cfg"] = Cc[k]
        m["cnt_ast"] = cnta[k]
        m["cnt_cfg"] = cntc[k]
        m["semT"] = np.ascontiguousarray(semT[:, k * cfg.GB:(k + 1) * cfg.GB])
        in_maps.append(m)
    return in_maps


def postprocess(results, cfg: CFG):
    outs = [np.asarray(results[k]["outT"]) for k in range(cfg.NCORES)]
    return np.concatenate(outs, axis=1).T.copy()  # [G, 2]


_CACHED = {}


def kernel(**inputs):
    from concourse.bass_utils import run_bass_kernel_spmd
    cfg = CFG()
    if "nc" not in _CACHED:
        _CACHED["nc"] = build_nc(cfg)
    in_maps = preprocess(inputs, cfg)
    res = run_bass_kernel_spmd(_CACHED["nc"], in_maps,
                               core_ids=list(range(cfg.NCORES)))
    return postprocess(res.results, cfg)

